# revision 24
# baseline (speedup 1.0000x reference)
"""MultiHeadAttention Trainium2 kernel.

Full inputs: x [4, 2048, 768] f32, W_qkv [2304, 768], W_proj [768, 768],
b_proj [768]. Output [4, 2048, 768] f32.

Sharding: 8 cores = 4 batches x 2 head-groups (6 heads each).
Per-core inputs (host-prepared, transposed on host):
  xT  [768, 2048]  = x[b].T
  wT  [768, 1152]  = concat(Wq_g, Wk_g, Wv_g).T   (g = head group rows)
  wpT [384, 768]   = W_proj[:, g-cols].T
Per-core output: outp [2048, 768] = partial projection output for batch b.
Host: out[b] = outp[2b] + outp[2b+1] + b_proj.

Key optimizations vs the 332.9us baseline (now ~272us):
  - QK matmuls run as row-tiled pairs (tile_position inferred from base
    partitions 0:64 / 64:128): two K=64 matmuls execute concurrently in
    the PE array, halving QK cost. kT_sb stores head pairs like qT_sb.
  - The softmax exp is split between the Scalar engine (real Exp LUT, odd
    kk chunks) and the Vector engine (even chunks; Schraudolph bit-trick:
    y = E*(128/ln2/8) + bias + 2^23 in f32 — the low 16 bits of y's
    mantissa ARE the bf16 of e^E; the AV matmul reads them as a stride-2
    bf16 view). One chunk covers both heads of a kk step. exp was the
    single biggest engine load (216us on ACT alone in the baseline).
  - The AV stationary carries the 64 v columns PLUS 64 ones columns, so
    av psum rows 64:128 hold the softmax denominator l replicated
    64-wide. One copy + one reciprocal_approx_fast [64,512] then yield
    the 1/l broadcast tile directly — no GPSIMD partition_broadcast, no
    single-lane full-precision reciprocals (79.5us in the baseline).
  - Normalize work (3 ACT copies + 1 DVE copy + 2 DVE recips + 2 GPSIMD
    multiplies per unit) is spread into the next unit's first kk slots so
    neither exp engine ever sees a boundary burst.
  - Phase 2 is software-pipelined 8 deep: AV for chunk kk issues after
    exp(kk+8), covering the exp latency at the PE period; the phase-2
    steady state is limited by the 3-deep e2 psum ring (PSUM has 8 banks:
    2 av accumulators + 3x2 energy tiles) and semaphore hop latency.
  - Phase 1 computes each (q/k, pair, n-half) with 4 concurrent psum
    accumulators and c-outer loops so consecutive matmuls reuse the
    stationary operand; psum->SBUF drains alternate ACT/DVE.
  - wp (phase-3 weights) DMA is deferred past the x/w input DMAs.
"""

import ml_dtypes
import numpy as np

import concourse.bass as bass
import concourse.tile as tile
from concourse import bacc, mybir
from concourse.bass_utils import run_bass_kernel_spmd

EMB = 768
N = 2048
B = 4
D = 64
HL = 6            # heads per core
HD = HL * D       # 384 local head-dim columns
NCORES = 8
SCALE = D ** -0.5

F32 = mybir.dt.float32
BF16 = mybir.dt.bfloat16
I16 = mybir.dt.int16

EC = EMB // 128   # 6 emb chunks
MC = HD // 128    # 3 head pairs
NQ = N // 512     # 4 query chunks of 512
NK = N // 128     # 16 key chunks of 128
DEPTH = 8         # AV software-pipeline depth (in kk steps)

EXP = mybir.ActivationFunctionType.Exp
MULT = mybir.AluOpType.mult
ADD = mybir.AluOpType.add

ASC = float(128.0 / np.log(2) * SCALE)      # schraudolph slope (scale folded)
BMAGIC = float(16250.5 + 2 ** 23)           # schraudolph bias + f32 round trick


def _emit(tc):
    from contextlib import ExitStack

    nc = tc.nc
    xT = nc.dram_tensor("xT", [EMB, N], BF16, kind="ExternalInput").ap()
    wT = nc.dram_tensor("wT", [EMB, 3 * HD], BF16, kind="ExternalInput").ap()
    wpT = nc.dram_tensor("wpT", [HD, EMB], BF16, kind="ExternalInput").ap()
    outp = nc.dram_tensor("outp", [N, EMB], F32, kind="ExternalOutput").ap()

    xTr = xT.rearrange("(c p) s -> p c s", p=128)
    wTr = wT.rearrange("(c p) s -> p c s", p=128)
    wpTr = wpT.rearrange("(m p) e -> p m e", p=128)
    outr = outp.rearrange("(s p) e -> p s e", p=128)

    with ExitStack() as persist:
        ppool = persist.enter_context(tc.tile_pool(name="persist", bufs=1))
        # PE warmup: junk matmuls run during the input-DMA wait to open the
        # HAM clock gate
        warm_sb = ppool.tile([128, 640], BF16)
        nc.vector.memset(warm_sb[:], 1.0)
        wp_sb = ppool.tile([128, MC, EMB], BF16)
        qT_sb = ppool.tile([128, MC, N], BF16)
        kT_sb = ppool.tile([128, MC, N], BF16)
        # per head block: [v columns (64) | ones columns (64)] so the AV
        # matmul also produces l replicated across 64 psum rows
        v_sb = ppool.tile([128, NK, HL * 2 * D], BF16)
        nc.vector.memset(
            v_sb[:].rearrange("p k (h c) -> p k h c", c=2 * D)[:, :, :, D:2 * D],
            1.0)
        attT_sb = ppool.tile([128, MC, N], BF16)

        psum_pool = persist.enter_context(
            tc.tile_pool(name="psum", bufs=1, space="PSUM"))
        warm_ps = psum_pool.tile([128, 512], F32, tag="av", bufs=2, name="warm_ps")
        for wi in range(10):
            nc.tensor.matmul(warm_ps[:], warm_sb[:, 0:128], warm_sb[:, 128:640],
                             start=(wi == 0), stop=(wi == 9))

        # ---- phases 1+2 share a scope: pair 1/2 qk projections are deferred
        # into phase-2 unit boundaries (PE absorbs them while the exp
        # engines catch up; x/w stay resident until they are done) ----
        with ExitStack() as ph2:
            p1 = ph2.enter_context(tc.tile_pool(name="ph1", bufs=1))
            x_sb = p1.tile([128, EC, N], BF16)
            w_sb = p1.tile([128, EC, 3 * HD], BF16)
            for c in range(EC):
                nc.sync.dma_start(w_sb[:, c, :], wTr[:, c, :])
                nc.sync.dma_start(x_sb[:, c, :], xTr[:, c, :])
            # wp is only needed in phase 3; don't put it ahead of x/w
            nc.sync.dma_start(wp_sb[:], wpTr)

            # phase 1: interleave the (LDW-serialized) v matmuls into the
            # q/k groups so their weight loads hide under q/k streaming.
            v_steps = []
            v_state = {}

            def v_step(s, c):
                if s not in v_state:
                    v_state[s] = psum_pool.tile([128, 2, 512], F32, tag="eps",
                                                bufs=3, name=f"vv_{s}")[:, 0, 0:HD]
                nc.tensor.matmul(
                    v_state[s],
                    (x_sb[:, c, s * 128:(s + 1) * 128]),
                    (w_sb[:, c, 2 * HD:3 * HD]),
                    start=(c == 0), stop=(c == EC - 1))
                if c == EC - 1:
                    nc.vector.tensor_copy(
                        v_sb[:, s, :].rearrange(
                            "p (h c) -> p h c", c=2 * D)[:, :, 0:D],
                        v_state.pop(s)[:].rearrange("p (h d) -> p h d", h=HL))

            for s in range(NK):
                for c in range(EC):
                    v_steps.append((s, c))

            def qk_block(which, m, nh, eps_bufs=3, interleave=False):
                # one (q-or-k, pair, n-half) projection: 12 matmuls + 2 copies
                lo = which * HD + m * 128
                mm4 = psum_pool.tile([128, 2, 512], F32, tag="eps",
                                     bufs=eps_bufs,
                                     name=f"mm4i_{which}_{m}_{nh}")
                for c in range(EC):
                    for j in (0, 1):
                        n = 2 * nh + j
                        nc.tensor.matmul(
                            mm4[:, j, :],
                            (w_sb[:, c, lo:lo + 128]),
                            (x_sb[:, c, n * 512:(n + 1) * 512]),
                            start=(c == 0), stop=(c == EC - 1))
                    if interleave:
                        for _ in range(2 if v_steps and len(v_steps) % 3 else 1):
                            if v_steps:
                                v_step(*v_steps.pop(0))
                dst = qT_sb if which == 0 else kT_sb
                for j in (0, 1):
                    n = 2 * nh + j
                    ns = slice(n * 512, (n + 1) * 512)
                    if (which + n) % 2 == 0:
                        nc.scalar.copy(dst[:, m, ns], mm4[:, j, :])
                    else:
                        nc.vector.tensor_copy(dst[:, m, ns], mm4[:, j, :])

            for which in (0, 1):
                for m in range(MC):
                    for nh in (0, 1):
                        qk_block(which, m, nh, interleave=True)
            while v_steps:
                v_step(*v_steps.pop(0))

            esb_pool = ph2.enter_context(tc.tile_pool(name="esb", bufs=4))
            sm_pool = ph2.enter_context(tc.tile_pool(name="sm", bufs=4))

            pending_norm = {}
            for m in range(MC):
                for n in range(NQ):
                    unit = m * NQ + n
                    ns = slice(n * 512, (n + 1) * 512)
                    kslice = lambda kk: slice(kk * 128, (kk + 1) * 128)
                    av_t = [psum_pool.tile([128, 512], F32, tag="av", bufs=2,
                                           name=f"av_{m}_{n}_{z}")
                            for z in (0, 1)]
                    mvq = []

                    def emit_av(j):
                        for z in (0, 1):
                            h = 2 * m + z
                            nc.tensor.matmul(
                                av_t[z][:],
                                (v_sb[:, j, h * 2 * D:(h + 1) * 2 * D]),
                                mvq[j][z],
                                start=(j == 0), stop=(j == NK - 1))

                    for kk in range(NK):
                        e2 = psum_pool.tile([128, 2, 512], F32, tag="eps",
                                            bufs=3, name=f"e_{m}_{n}_{kk}")
                        nc.tensor.matmul(e2[:, 0, :],
                                         (kT_sb[0:64, m, kslice(kk)]),
                                         (qT_sb[0:64, m, ns]),
                                         start=True, stop=True)
                        nc.tensor.matmul(e2[:, 1, :],
                                         (kT_sb[64:128, m, kslice(kk)]),
                                         (qT_sb[64:128, m, ns]),
                                         start=True, stop=True)
                        if kk >= DEPTH:
                            emit_av(kk - DEPTH)
                        if kk in pending_norm:
                            for fn in pending_norm.pop(kk):
                                fn()
                        if kk % 2 == 1:
                            esb = esb_pool.tile([128, 2, 512], BF16, tag="esb",
                                                bufs=10, name=f"esb_{m}_{n}_{kk}")
                            nc.scalar.activation(esb[:], e2[:], EXP, scale=SCALE)
                            mvq.append((esb[:, 0, :], esb[:, 1, :]))
                        else:
                            esf = esb_pool.tile([128, 2, 512], F32, tag="esf",
                                                bufs=10, name=f"esf_{m}_{n}_{kk}")
                            nc.vector.tensor_scalar(esf[:], e2[:], ASC, BMAGIC,
                                                    MULT, ADD)
                            bv = esf[:].bitcast(I16)[:, :, 0::2].bitcast(BF16)
                            mvq.append((bv[:, 0, :], bv[:, 1, :]))
                    for j in range(NK - DEPTH, NK):
                        emit_av(j)

                    # drain + normalize for this (pair, n):
                    # rows 0:64 = av, rows 64:128 = l replicated 64-wide.
                    # copies split 3 ACT / 1 DVE; 1/l on DVE; the attT
                    # multiplies run on GPSIMD (own queue, off the exp path).
                    # All ops are spread into the next unit's first kk slots
                    # so neither engine sees a boundary burst.
                    tiles = []
                    for z in (0, 1):
                        tiles.append((
                            sm_pool.tile([D, 512], F32, tag=f"avst{z}",
                                         bufs=3, name=f"avst_{m}_{n}_{z}"),
                            sm_pool.tile([D, 512], F32, tag=f"lrep{z}",
                                         bufs=3, name=f"lrep_{m}_{n}_{z}"),
                            sm_pool.tile([D, 512], F32, tag=f"rb{z}",
                                         bufs=3, name=f"rb_{m}_{n}_{z}")))

                    def norm_ops(m=m, ns=ns, av_t=av_t, tiles=tiles):
                        a0, l0, r0 = tiles[0]
                        a1, l1, r1 = tiles[1]

                        def mul(z, a, r):
                            nc.gpsimd.tensor_mul(
                                attT_sb[z * 64:(z + 1) * 64, m, ns], a[:], r[:])
                        return {
                            1: [lambda: nc.scalar.copy(a0[:], av_t[0][0:D, :])],
                            2: [lambda: nc.scalar.copy(a1[:], av_t[1][0:D, :]),
                                lambda: nc.vector.tensor_copy(
                                    l0[:], av_t[0][D:2 * D, :])],
                            3: [lambda: nc.scalar.copy(l1[:],
                                                       av_t[1][D:2 * D, :]),
                                lambda: nc.vector.reciprocal_approx_fast(
                                    r0[:], l0[:]),
                                lambda: mul(0, a0, r0)],
                            4: [lambda: nc.vector.reciprocal_approx_fast(
                                    r1[:], l1[:]),
                                lambda: mul(1, a1, r1)],
                        }

                    if unit == MC * NQ - 1:
                        for kk, fns in sorted(norm_ops().items()):
                            for fn in fns:
                                fn()
                    else:
                        pending_norm = norm_ops()

        # keep the PE array busy through the last unit's normalization tail
        fill_ps = psum_pool.tile([128, 512], F32, tag="av", bufs=2, name="fill_ps")
        for wi in range(16):
            nc.tensor.matmul(fill_ps[:], warm_sb[:, 0:128], warm_sb[:, 128:640],
                             start=(wi == 0), stop=(wi == 15))

        # ---- phase 3: output projection ----
        with ExitStack() as ph3:
            osb_pool = ph3.enter_context(tc.tile_pool(name="osb", bufs=3))
            for s in range(NK):
                o_sb = osb_pool.tile([128, EMB], F32, tag="osb", name=f"osb_{s}")
                for half in range(2):
                    pr = psum_pool.tile([128, 512], F32, tag="av", bufs=2,
                                        name=f"pr_{s}_{half}")[:, 0:HD]
                    for m in range(MC):
                        nc.tensor.matmul(
                            pr[:],
                            (attT_sb[:, m, s * 128:(s + 1) * 128]),
                            (wp_sb[:, m, half * HD:(half + 1) * HD]),
                            start=(m == 0), stop=(m == MC - 1))
                    if half == 0:
                        nc.vector.tensor_copy(o_sb[:, 0:HD], pr[:])
                    else:
                        nc.scalar.copy(o_sb[:, HD:2 * HD], pr[:])
                nc.sync.dma_start(outr[:, s, :], o_sb[:])


_CACHE = {}


def _build():
    if "nc" not in _CACHE:
        nc = bacc.Bacc("TRN2", target_bir_lowering=False, debug=False,
                       num_devices=NCORES)
        with tile.TileContext(nc) as tc:
            _emit(tc)
        nc.compile()
        _CACHE["nc"] = nc
    return _CACHE["nc"]


def _in_maps(x, W_qkv, W_proj):
    in_maps = []
    for c in range(NCORES):
        b, g = divmod(c, 2)
        r0 = g * HD
        w_rows = np.concatenate([
            W_qkv[0 * EMB + r0: 0 * EMB + r0 + HD],
            W_qkv[1 * EMB + r0: 1 * EMB + r0 + HD],
            W_qkv[2 * EMB + r0: 2 * EMB + r0 + HD],
        ], axis=0)                                   # [1152, 768]
        bf = ml_dtypes.bfloat16
        in_maps.append({
            "xT": np.ascontiguousarray(x[b].T.astype(bf)),
            "wT": np.ascontiguousarray(w_rows.T.astype(bf)),
            "wpT": np.ascontiguousarray(W_proj[:, r0:r0 + HD].T.astype(bf)),
        })
    return in_maps


LAST_RESULTS = None


def kernel(x, W_qkv, W_proj, b_proj):
    global LAST_RESULTS
    x = np.ascontiguousarray(np.asarray(x, dtype=np.float32))
    W_qkv = np.asarray(W_qkv, dtype=np.float32)
    W_proj = np.asarray(W_proj, dtype=np.float32)
    b_proj = np.asarray(b_proj, dtype=np.float32)

    nc = _build()
    in_maps = _in_maps(x, W_qkv, W_proj)
    res = run_bass_kernel_spmd(nc, in_maps, core_ids=list(range(NCORES)))
    LAST_RESULTS = res

    out = np.empty((B, N, EMB), dtype=np.float32)
    for b in range(B):
        out[b] = res.results[2 * b]["outp"] + res.results[2 * b + 1]["outp"]
    out += b_proj
    return out
